# revision 16
# baseline (speedup 1.0000x reference)
"""NNConv (gnn_message_passing) Bass kernel for 8 Trainium2 NeuronCores.

Strategy (edge-parallel, dst-sharded):
- Host relabels nodes with a permutation so that the 16384 nodes form 128
  "windows" of 128 nodes, each window receiving exactly 512 edges (by
  destination).  Core c owns windows [16c, 16c+16): 2048 nodes / 8192 edges.
  This makes the device program identical across cores (pure SPMD); all
  per-core variation lives in the input data.
- Per 128-edge tile, on device:
    P   = attr_aug^T @ Aaug            (PE matmul -> PSUM, f32)
    q   = relu(P) * x[src] broadcast   (one fused DVE scalar_tensor_tensor,
                                        bf16 out; x gathered by indirect DMA)
    agg += onehot(dst)^T @ q           (PE matmul; zero-step output AP sums
                                        the c_in axis into the same PSUM
                                        columns while contracting edges)
  The root term (x @ root) and a per-window one-hot matmul accumulate into
  the same PSUM bank; bias is added during the PSUM->SBUF copy.
- h1 is exchanged between layers with an AllGather (bf16).
"""

import numpy as np
import ml_dtypes
from contextlib import ExitStack

import concourse.bass as bass
import concourse.tile as tile
from concourse import bacc, mybir
from concourse.bass import IndirectOffsetOnAxis
from concourse.bass_utils import run_bass_kernel_spmd

dt = mybir.dt
BF16 = ml_dtypes.bfloat16

N = 16384
E = 65536
NCORES = 8
P = 128                 # partitions / edges per tile
WINDOWS = 128           # global 128-node windows
WPC = WINDOWS // NCORES  # 16 windows per core
NPC = N // NCORES        # 2048 nodes per core
COUT = 64
CIN1 = 8
CIN2 = 64
ICH = 8                  # c_in values per P-gen chunk (chunk = ICH*COUT cols)
NALT = 0                 # layer-2 chunks routed Act-relu/Pool-mult/DVE-reduce
IALT = CIN2 - ICH * NALT  # first i covered by the alt route (32)
OBL = (ICH * COUT) // (CIN2 - IALT) if NALT else 0  # o-values per alt chunk

_cached = {}


def _build_program(U):
    """Build the SPMD Bass program. U = edges per window (multiple of 128)."""
    T = U // P  # tiles per window
    EPC = WPC * U  # edges per core

    nc = bacc.Bacc("TRN2", target_bir_lowering=False, debug=False,
                   num_devices=NCORES)

    attrT_d = nc.dram_tensor("attrT", [3, EPC], dt.bfloat16, kind="ExternalInput").ap()
    src_d = nc.dram_tensor("srcidx", [EPC, 1], dt.int32, kind="ExternalInput").ap()
    dstrel_d = nc.dram_tensor("dstrel", [EPC, 1], dt.float32, kind="ExternalInput").ap()
    A1aug_d = nc.dram_tensor("A1aug", [3, CIN1 * COUT], dt.bfloat16, kind="ExternalInput").ap()
    A2aug_d = nc.dram_tensor("A2aug", [3, CIN2 * COUT], dt.bfloat16, kind="ExternalInput").ap()
    xg1_d = nc.dram_tensor("xg1", [EPC, CIN1], dt.bfloat16, kind="ExternalInput").ap()
    xT_d = nc.dram_tensor("xT", [CIN1, NPC], dt.bfloat16, kind="ExternalInput").ap()
    r1_d = nc.dram_tensor("root1", [CIN1, COUT], dt.bfloat16, kind="ExternalInput").ap()
    r2_d = nc.dram_tensor("root2", [CIN2, COUT], dt.bfloat16, kind="ExternalInput").ap()
    b1_d = nc.dram_tensor("bias1", [P, COUT], dt.float32, kind="ExternalInput").ap()
    b2_d = nc.dram_tensor("bias2", [P, COUT], dt.float32, kind="ExternalInput").ap()
    iota_d = nc.dram_tensor("iota", [P, P], dt.bfloat16, kind="ExternalInput").ap()
    out_d = nc.dram_tensor("out", [NPC, COUT], dt.float32, kind="ExternalOutput").ap()

    with tile.TileContext(nc) as tc, ExitStack() as ctx, \
            nc.allow_low_precision("bf16 msum; abs tolerance 2e-2"):
        consts = ctx.enter_context(tc.tile_pool(name="consts", bufs=1))
        meta = ctx.enter_context(tc.tile_pool(name="meta", bufs=8))
        xgp = ctx.enter_context(tc.tile_pool(name="xgp", bufs=6))
        ohp = ctx.enter_context(tc.tile_pool(name="ohp", bufs=6))
        qp = ctx.enter_context(tc.tile_pool(name="qp", bufs=6))
        rp = ctx.enter_context(tc.tile_pool(name="rp", bufs=4))
        msp = ctx.enter_context(tc.tile_pool(name="msp", bufs=4))
        rootp = ctx.enter_context(tc.tile_pool(name="rootp", bufs=1))
        hp = ctx.enter_context(tc.tile_pool(name="hp", bufs=1))
        outp = ctx.enter_context(tc.tile_pool(name="outp", bufs=3))
        pp = ctx.enter_context(tc.tile_pool(name="pp", bufs=2, space="PSUM"))
        aggp = ctx.enter_context(tc.tile_pool(name="aggp", bufs=4, space="PSUM"))
        dramp = ctx.enter_context(tc.tile_pool(name="dram", bufs=1, space="DRAM"))

        A1_s = consts.tile([3, CIN1 * COUT], dt.bfloat16)
        nc.sync.dma_start(A1_s[:], A1aug_d[:])
        A2_s = consts.tile([3, CIN2 * COUT], dt.bfloat16)
        nc.sync.dma_start(A2_s[:], A2aug_d[:])
        iota_s = consts.tile([P, P], dt.bfloat16)
        nc.sync.dma_start(iota_s[:], iota_d[:])
        r1_s = consts.tile([CIN1, COUT], dt.bfloat16)
        nc.sync.dma_start(r1_s[:], r1_d[:])
        r2_s = consts.tile([CIN2, COUT], dt.bfloat16)
        nc.sync.dma_start(r2_s[:], r2_d[:])
        b1_s = consts.tile([P, COUT], dt.float32)
        nc.sync.dma_start(b1_s[:], b1_d[:])
        b2_s = consts.tile([P, COUT], dt.float32)
        nc.sync.dma_start(b2_s[:], b2_d[:])

        # h1 slice (local) and allgathered h1 (global), bf16
        hloc = dramp.tile([NPC, COUT], dt.bfloat16)
        hglob = dramp.tile([N, COUT], dt.bfloat16)

        def layer(cin, A_s, is_l1, lhsT_list):
            nchunks = cin // ICH
            for w in range(WPC):
                aggw = aggp.tile([P, COUT], dt.float32, tag="aggw")
                # root-term matmul opens the accumulation (start=True);
                # the lhsT tiles were loaded up front (overlapped with the
                # previous phase)
                nc.tensor.matmul(aggw[:], lhsT=lhsT_list[w][:],
                                 rhs=r1_s[:] if is_l1 else r2_s[:],
                                 start=True, stop=False)
                for t in range(T):
                    e0 = (w * T + t) * P
                    attr_t = meta.tile([3, P], dt.bfloat16, tag="attr")
                    nc.sync.dma_start(attr_t[:], attrT_d[:, e0:e0 + P])
                    dstt = meta.tile([P, 1], dt.float32, tag="dst")
                    nc.sync.dma_start(dstt[:], dstrel_d[e0:e0 + P, :])

                    cin_t = CIN1 if is_l1 else CIN2
                    xg = xgp.tile([P, cin_t], dt.bfloat16,
                                  tag="xg1" if is_l1 else "xg2")
                    if is_l1:
                        nc.sync.dma_start(xg[:], xg1_d[e0:e0 + P, :])
                    else:
                        srct = meta.tile([P, 1], dt.int32, tag="src")
                        nc.sync.dma_start(srct[:], src_d[e0:e0 + P, :])
                        nc.gpsimd.indirect_dma_start(
                            out=xg[:], out_offset=None, in_=hglob[:],
                            in_offset=IndirectOffsetOnAxis(ap=srct[:, :1], axis=0))

                    oh = ohp.tile([P, P], dt.bfloat16, tag="oh")
                    nc.gpsimd.tensor_scalar(
                        out=oh[:], in0=iota_s[:], scalar1=dstt[:, :1],
                        scalar2=None, op0=mybir.AluOpType.is_equal)

                    # layer 2: chunks 4-7 take the Act/Pool/DVE-reduce
                    # route ((o,i)-permuted A columns, i in [32,64))
                    nalt = 0 if is_l1 else NALT
                    if nalt:
                        msum = msp.tile([P, COUT], dt.bfloat16, tag="ms")
                    ich = min(cin, 2 * ICH)  # 8 (l1) or 16 (l2)
                    ngrp = cin // ich
                    for c in range(ngrp):
                        cols = ich * COUT
                        ppc = pp.tile([P, 2 * ICH * COUT], dt.float32,
                                      tag="ppc")
                        for h in range(0, cols, 512):
                            nc.tensor.matmul(
                                ppc[:, h:h + 512], lhsT=attr_t[:],
                                rhs=A_s[:, c * cols + h:c * cols + h + 512],
                                start=True, stop=True)
                        if True:
                            qc = qp.tile([P, cols], dt.bfloat16,
                                         tag="qc1" if is_l1 else "qc2")
                            q3 = qc[:].rearrange("p (i o) -> p i o", i=ich)
                            nc.vector.scalar_tensor_tensor(
                                out=q3, in0=ppc[:, :cols].rearrange("p (i o) -> p i o", i=ich),
                                scalar=0.0,
                                in1=xg[:, c * ich:(c + 1) * ich].to_broadcast(
                                    [P, ich, COUT]),
                                op0=mybir.AluOpType.max, op1=mybir.AluOpType.mult)
                            for h in range(0, cols, 512):
                                ii = 512 // COUT
                                nc.tensor.matmul(
                                    aggw[:].unsqueeze(1).broadcast_to([P, ii, COUT]),
                                    lhsT=oh[:],
                                    rhs=qc[:, h:h + 512].rearrange(
                                        "p (i o) -> p i o", i=ii),
                                    start=False,
                                    stop=(t == T - 1 and c == ngrp - 1
                                          and h + 512 >= cols and nalt == 0),
                                    skip_group_check=True)
                        else:
                            # (o,i) chunk: o-block of OBL, i in [IALT, 64)
                            cb = c - (nchunks - nalt)
                            ialt = CIN2 - ICH * nalt
                            rc = rp.tile([P, cols], dt.bfloat16, tag="rc")
                            nc.scalar.activation(
                                out=rc[:], in_=ppc[:],
                                func=mybir.ActivationFunctionType.Relu)
                            r3 = rc[:].rearrange("p (o i) -> p o i", o=OBL)
                            qc = qp.tile([P, cols], dt.bfloat16, tag="qa")
                            q3 = qc[:].rearrange("p (o i) -> p o i", o=OBL)
                            nc.gpsimd.tensor_tensor(
                                out=q3, in0=r3,
                                in1=xg[:, ialt:].unsqueeze(1).broadcast_to(
                                    [P, OBL, CIN2 - ialt]),
                                op=mybir.AluOpType.mult)
                            nc.vector.tensor_reduce(
                                out=msum[:, cb * OBL:(cb + 1) * OBL], in_=q3,
                                axis=mybir.AxisListType.X, op=mybir.AluOpType.add)
                    if nalt:
                        nc.tensor.matmul(aggw[:], lhsT=oh[:], rhs=msum[:],
                                         start=False, stop=(t == T - 1),
                                         skip_group_check=True)
                # finalize window: add bias, write out
                if is_l1:
                    hw_ = outp.tile([P, COUT], dt.bfloat16, tag="h1w")
                    nc.vector.tensor_tensor(out=hw_[:], in0=aggw[:], in1=b1_s[:],
                                            op=mybir.AluOpType.add)
                    nc.sync.dma_start(hloc[w * P:(w + 1) * P, :], hw_[:])
                else:
                    ow = outp.tile([P, COUT], dt.float32, tag="outw")
                    nc.vector.tensor_tensor(out=ow[:], in0=aggw[:], in1=b2_s[:],
                                            op=mybir.AluOpType.add)
                    nc.sync.dma_start(out_d[w * P:(w + 1) * P, :], ow[:])

        roots1 = []
        for w in range(WPC):
            lhsTw = rootp.tile([CIN1, P], dt.bfloat16, tag=f"rootl1_{w}")
            nc.sync.dma_start(lhsTw[:], xT_d[:, w * P:(w + 1) * P])
            roots1.append(lhsTw)
        layer(CIN1, A1_s, True, roots1)
        # layer-2 root lhsT transposes depend only on the LOCAL h1 slice, so
        # they overlap the layer-1 tail and the AllGather
        roots2 = []
        for w in range(WPC):
            lhsTw = rootp.tile([CIN2, P], dt.bfloat16, tag=f"rootl2_{w}")
            nc.sync.dma_start_transpose(lhsTw[:], hloc[w * P:(w + 1) * P, :])
            roots2.append(lhsTw)
        nc.gpsimd.collective_compute(
            "AllGather", mybir.AluOpType.bypass,
            replica_groups=[list(range(NCORES))],
            ins=[hloc[:].opt()], outs=[hglob[:].opt()])
        layer(CIN2, A2_s, False, roots2)

    nc.compile()
    return nc


def _pack(edge_index):
    """Relabel nodes into 128 windows of 128 nodes / exactly U edges each.

    Returns (perm, U, order) where perm[orig_node] = new node id and
    order = edge permutation grouping edges by destination window, padded.
    """
    dst = np.asarray(edge_index[1], dtype=np.int64)
    deg = np.bincount(dst, minlength=N).astype(np.int64)
    # LPT greedy: descending degree, least-loaded window with free slots
    nodes = np.argsort(-deg, kind="stable")
    loads = np.zeros(WINDOWS, dtype=np.int64)
    slots = np.zeros(WINDOWS, dtype=np.int64)
    wof = np.empty(N, dtype=np.int64)  # window of node
    for v in nodes:
        open_w = np.flatnonzero(slots < P)
        w = open_w[np.argmin(loads[open_w])]
        wof[v] = w
        loads[w] += deg[v]
        slots[w] += 1
    # repair toward exact target load by swapping nodes between windows
    target = E // WINDOWS
    if loads.max() > target:
        by_wd = {}  # (window, degree) -> list of nodes
        for v in range(N):
            by_wd.setdefault((wof[v], deg[v]), []).append(v)
        for _ in range(100000):
            over = int(np.argmax(loads))
            under = int(np.argmin(loads))
            if loads[over] <= target:
                break
            delta = min(loads[over] - target, target - loads[under])
            # find a swap pair with degree difference = d, largest d first
            done = False
            for d in range(int(delta), 0, -1):
                for da in range(int(deg.max()), d - 1, -1):
                    la = by_wd.get((over, da))
                    lb = by_wd.get((under, da - d))
                    if la and lb:
                        a, b = la.pop(), lb.pop()
                        wof[a], wof[b] = under, over
                        by_wd.setdefault((under, da), []).append(a)
                        by_wd.setdefault((over, da - d), []).append(b)
                        loads[over] -= d
                        loads[under] += d
                        done = True
                        break
                if done:
                    break
            if not done:
                break
    U = int(np.ceil(loads.max() / P) * P)
    # perm: nodes sorted by window -> new ids
    new_order = np.argsort(wof * N + np.arange(N), kind="stable")
    perm = np.empty(N, dtype=np.int64)
    perm[new_order] = np.arange(N)
    # edge order: group by destination window, pad each window to U
    ew = wof[dst]
    eorder = np.argsort(ew, kind="stable")
    counts = np.bincount(ew, minlength=WINDOWS)
    padded = np.full(WINDOWS * U, -1, dtype=np.int64)
    pos = 0
    for w in range(WINDOWS):
        c = int(counts[w])
        padded[w * U:w * U + c] = eorder[pos:pos + c]
        pos += c
    return perm, U, padded


def kernel(x, edge_index, edge_attr, A1, b1, A2, b2, root1, bias1, root2, bias2):
    x = np.asarray(x, dtype=np.float32)
    edge_index = np.asarray(edge_index)
    edge_attr = np.asarray(edge_attr, dtype=np.float32)

    perm, U, padded = _pack(edge_index)
    key = U
    if key not in _cached:
        _cached[key] = _build_program(U)
    nc = _cached[key]

    src = np.asarray(edge_index[0], dtype=np.int64)
    dst = np.asarray(edge_index[1], dtype=np.int64)
    valid = padded >= 0
    pe = np.where(valid, padded, 0)
    # per padded-edge data
    a01 = edge_attr[pe]                      # [W*U, 2]
    aug = valid.astype(np.float32)
    attrT_all = np.stack([a01[:, 0] * aug, a01[:, 1] * aug, aug]).astype(BF16)  # [3, W*U]
    srcn_all = np.where(valid, perm[src[pe]], 0).astype(np.int32)
    dstn = perm[dst[pe]]
    wof_e = np.arange(WINDOWS).repeat(U)
    dstrel_all = np.where(valid, dstn - wof_e * P, 0).astype(np.float32)

    x_pi = np.empty_like(x)
    x_pi[perm] = x
    x_bf = x_pi.astype(BF16)
    xg1_all = x_bf[srcn_all]                 # host pre-gather for layer 1

    A1aug = np.concatenate([A1, b1[None, :]], axis=0).astype(BF16)
    A2aug = np.concatenate([A2, b2[None, :]], axis=0).astype(BF16)
    # alt-route chunks cb cover o in [16*cb, 16*cb+16), i in [IALT, 64),
    # laid out o-major / i-minor
    cols = ICH * COUT
    j = np.arange(NALT * cols)
    cb = j // cols
    rem = j % cols
    ol = rem // (CIN2 - IALT)
    i = IALT + rem % (CIN2 - IALT)
    src_cols = i * COUT + (OBL * cb + ol)
    A2perm = A2aug.copy()
    A2perm[:, (8 - NALT) * cols:] = A2aug[:, src_cols]
    iota_np = np.broadcast_to(np.arange(P, dtype=np.float32), (P, P)).astype(BF16)
    b1_bc = np.broadcast_to(bias1, (P, COUT)).astype(np.float32).copy()
    b2_bc = np.broadcast_to(bias2, (P, COUT)).astype(np.float32).copy()
    shared = {
        "A1aug": A1aug, "A2aug": A2perm,
        "root1": np.asarray(root1.astype(BF16)),
        "root2": np.asarray(root2.astype(BF16)),
        "bias1": b1_bc, "bias2": b2_bc,
        "iota": np.asarray(iota_np),
    }
    EPC = WPC * U
    in_maps = []
    for c in range(NCORES):
        s = c * EPC
        m = dict(shared)
        m["attrT"] = attrT_all[:, s:s + EPC].copy()
        m["srcidx"] = srcn_all[s:s + EPC].reshape(EPC, 1).copy()
        m["dstrel"] = dstrel_all[s:s + EPC].reshape(EPC, 1).copy()
        m["xg1"] = np.ascontiguousarray(xg1_all[s:s + EPC])
        m["xT"] = np.ascontiguousarray(x_bf[c * NPC:(c + 1) * NPC].T)
        in_maps.append(m)

    res = run_bass_kernel_spmd(nc, in_maps, list(range(NCORES)),
                               **kernel.run_kwargs)
    kernel.last_result = res
    out_pi = np.concatenate([res.results[c]["out"] for c in range(NCORES)], axis=0)
    return out_pi[perm]


kernel.run_kwargs = {}
kernel.last_result = None



# revision 17
# speedup vs baseline: 1.0039x; 1.0039x over previous
"""NNConv (gnn_message_passing) Bass kernel for 8 Trainium2 NeuronCores.

Strategy (edge-parallel, dst-sharded):
- Host relabels nodes with a permutation so that the 16384 nodes form 128
  "windows" of 128 nodes, each window receiving exactly 512 edges (by
  destination).  Core c owns windows [16c, 16c+16): 2048 nodes / 8192 edges.
  This makes the device program identical across cores (pure SPMD); all
  per-core variation lives in the input data.
- Per 128-edge tile, on device:
    P   = attr_aug^T @ Aaug            (PE matmul -> PSUM, f32)
    q   = relu(P) * x[src] broadcast   (one fused DVE scalar_tensor_tensor,
                                        bf16 out; x gathered by indirect DMA)
    agg += onehot(dst)^T @ q           (PE matmul; zero-step output AP sums
                                        the c_in axis into the same PSUM
                                        columns while contracting edges)
  The root term (x @ root) and a per-window one-hot matmul accumulate into
  the same PSUM bank; bias is added during the PSUM->SBUF copy.
- h1 is exchanged between layers with an AllGather (bf16).
"""

import numpy as np
import ml_dtypes
from contextlib import ExitStack

import concourse.bass as bass
import concourse.tile as tile
from concourse import bacc, mybir
from concourse.bass import IndirectOffsetOnAxis
from concourse.bass_utils import run_bass_kernel_spmd

dt = mybir.dt
BF16 = ml_dtypes.bfloat16

N = 16384
E = 65536
NCORES = 8
P = 128                 # partitions / edges per tile
WINDOWS = 128           # global 128-node windows
WPC = WINDOWS // NCORES  # 16 windows per core
NPC = N // NCORES        # 2048 nodes per core
COUT = 64
CIN1 = 8
CIN2 = 64
ICH = 8                  # c_in values per P-gen chunk (chunk = ICH*COUT cols)
NALT = 0                 # layer-2 chunks routed Act-relu/Pool-mult/DVE-reduce
IALT = CIN2 - ICH * NALT  # first i covered by the alt route (32)
OBL = (ICH * COUT) // (CIN2 - IALT) if NALT else 0  # o-values per alt chunk

_cached = {}


def _build_program(U):
    """Build the SPMD Bass program. U = edges per window (multiple of 128)."""
    T = U // P  # tiles per window
    EPC = WPC * U  # edges per core

    nc = bacc.Bacc("TRN2", target_bir_lowering=False, debug=False,
                   num_devices=NCORES)

    attrT_d = nc.dram_tensor("attrT", [3, EPC], dt.bfloat16, kind="ExternalInput").ap()
    src_d = nc.dram_tensor("srcidx", [EPC, 1], dt.int32, kind="ExternalInput").ap()
    dstrel_d = nc.dram_tensor("dstrel", [EPC, 1], dt.float32, kind="ExternalInput").ap()
    A1aug_d = nc.dram_tensor("A1aug", [3, CIN1 * COUT], dt.bfloat16, kind="ExternalInput").ap()
    A2aug_d = nc.dram_tensor("A2aug", [3, CIN2 * COUT], dt.bfloat16, kind="ExternalInput").ap()
    xg1_d = nc.dram_tensor("xg1", [EPC, CIN1], dt.bfloat16, kind="ExternalInput").ap()
    xT_d = nc.dram_tensor("xT", [CIN1, NPC], dt.bfloat16, kind="ExternalInput").ap()
    r1_d = nc.dram_tensor("root1", [CIN1, COUT], dt.bfloat16, kind="ExternalInput").ap()
    r2_d = nc.dram_tensor("root2", [CIN2, COUT], dt.bfloat16, kind="ExternalInput").ap()
    b1_d = nc.dram_tensor("bias1", [P, COUT], dt.float32, kind="ExternalInput").ap()
    b2_d = nc.dram_tensor("bias2", [P, COUT], dt.float32, kind="ExternalInput").ap()
    iota_d = nc.dram_tensor("iota", [P, P], dt.bfloat16, kind="ExternalInput").ap()
    out_d = nc.dram_tensor("out", [NPC, COUT], dt.float32, kind="ExternalOutput").ap()

    with tile.TileContext(nc) as tc, ExitStack() as ctx, \
            nc.allow_low_precision("bf16 msum; abs tolerance 2e-2"):
        consts = ctx.enter_context(tc.tile_pool(name="consts", bufs=1))
        meta = ctx.enter_context(tc.tile_pool(name="meta", bufs=8))
        xgp = ctx.enter_context(tc.tile_pool(name="xgp", bufs=6))
        ohp = ctx.enter_context(tc.tile_pool(name="ohp", bufs=6))
        qp = ctx.enter_context(tc.tile_pool(name="qp", bufs=6))
        rp = ctx.enter_context(tc.tile_pool(name="rp", bufs=4))
        msp = ctx.enter_context(tc.tile_pool(name="msp", bufs=4))
        rootp = ctx.enter_context(tc.tile_pool(name="rootp", bufs=1))
        hp = ctx.enter_context(tc.tile_pool(name="hp", bufs=1))
        outp = ctx.enter_context(tc.tile_pool(name="outp", bufs=3))
        pp = ctx.enter_context(tc.tile_pool(name="pp", bufs=3, space="PSUM"))
        aggp = ctx.enter_context(tc.tile_pool(name="aggp", bufs=2, space="PSUM"))
        dramp = ctx.enter_context(tc.tile_pool(name="dram", bufs=1, space="DRAM"))

        A1_s = consts.tile([3, CIN1 * COUT], dt.bfloat16)
        nc.sync.dma_start(A1_s[:], A1aug_d[:])
        A2_s = consts.tile([3, CIN2 * COUT], dt.bfloat16)
        nc.sync.dma_start(A2_s[:], A2aug_d[:])
        iota_s = consts.tile([P, P], dt.bfloat16)
        nc.sync.dma_start(iota_s[:], iota_d[:])
        r1_s = consts.tile([CIN1, COUT], dt.bfloat16)
        nc.sync.dma_start(r1_s[:], r1_d[:])
        r2_s = consts.tile([CIN2, COUT], dt.bfloat16)
        nc.sync.dma_start(r2_s[:], r2_d[:])
        b1_s = consts.tile([P, COUT], dt.float32)
        nc.sync.dma_start(b1_s[:], b1_d[:])
        b2_s = consts.tile([P, COUT], dt.float32)
        nc.sync.dma_start(b2_s[:], b2_d[:])

        # h1 slice (local) and allgathered h1 (global), bf16
        hloc = dramp.tile([NPC, COUT], dt.bfloat16)
        hglob = dramp.tile([N, COUT], dt.bfloat16)

        def layer(cin, A_s, is_l1, lhsT_list):
            nchunks = cin // ICH
            for w in range(WPC):
                aggw = aggp.tile([P, COUT], dt.float32, tag="aggw")
                # root-term matmul opens the accumulation (start=True);
                # the lhsT tiles were loaded up front (overlapped with the
                # previous phase)
                nc.tensor.matmul(aggw[:], lhsT=lhsT_list[w][:],
                                 rhs=r1_s[:] if is_l1 else r2_s[:],
                                 start=True, stop=False)
                for t in range(T):
                    e0 = (w * T + t) * P
                    attr_t = meta.tile([3, P], dt.bfloat16, tag="attr")
                    nc.sync.dma_start(attr_t[:], attrT_d[:, e0:e0 + P])
                    dstt = meta.tile([P, 1], dt.float32, tag="dst")
                    nc.sync.dma_start(dstt[:], dstrel_d[e0:e0 + P, :])

                    cin_t = CIN1 if is_l1 else CIN2
                    xg = xgp.tile([P, cin_t], dt.bfloat16,
                                  tag="xg1" if is_l1 else "xg2")
                    if is_l1:
                        nc.sync.dma_start(xg[:], xg1_d[e0:e0 + P, :])
                    else:
                        srct = meta.tile([P, 1], dt.int32, tag="src")
                        nc.sync.dma_start(srct[:], src_d[e0:e0 + P, :])
                        nc.gpsimd.indirect_dma_start(
                            out=xg[:], out_offset=None, in_=hglob[:],
                            in_offset=IndirectOffsetOnAxis(ap=srct[:, :1], axis=0))

                    oh = ohp.tile([P, P], dt.bfloat16, tag="oh")
                    nc.gpsimd.tensor_scalar(
                        out=oh[:], in0=iota_s[:], scalar1=dstt[:, :1],
                        scalar2=None, op0=mybir.AluOpType.is_equal)

                    # layer 2: chunks 4-7 take the Act/Pool/DVE-reduce
                    # route ((o,i)-permuted A columns, i in [32,64))
                    nalt = 0 if is_l1 else NALT
                    if nalt:
                        msum = msp.tile([P, COUT], dt.bfloat16, tag="ms")
                    ich = min(cin, 2 * ICH)  # 8 (l1) or 16 (l2)
                    ngrp = cin // ich
                    for c in range(ngrp):
                        cols = ich * COUT
                        ppc = pp.tile([P, 2 * ICH * COUT], dt.float32,
                                      tag="ppc")
                        for h in range(0, cols, 512):
                            nc.tensor.matmul(
                                ppc[:, h:h + 512], lhsT=attr_t[:],
                                rhs=A_s[:, c * cols + h:c * cols + h + 512],
                                start=True, stop=True)
                        if True:
                            qc = qp.tile([P, cols], dt.bfloat16,
                                         tag="qc1" if is_l1 else "qc2")
                            q3 = qc[:].rearrange("p (i o) -> p i o", i=ich)
                            nc.vector.scalar_tensor_tensor(
                                out=q3, in0=ppc[:, :cols].rearrange("p (i o) -> p i o", i=ich),
                                scalar=0.0,
                                in1=xg[:, c * ich:(c + 1) * ich].to_broadcast(
                                    [P, ich, COUT]),
                                op0=mybir.AluOpType.max, op1=mybir.AluOpType.mult)
                            for h in range(0, cols, 512):
                                ii = 512 // COUT
                                nc.tensor.matmul(
                                    aggw[:].unsqueeze(1).broadcast_to([P, ii, COUT]),
                                    lhsT=oh[:],
                                    rhs=qc[:, h:h + 512].rearrange(
                                        "p (i o) -> p i o", i=ii),
                                    start=False,
                                    stop=(t == T - 1 and c == ngrp - 1
                                          and h + 512 >= cols and nalt == 0),
                                    skip_group_check=True)
                        else:
                            # (o,i) chunk: o-block of OBL, i in [IALT, 64)
                            cb = c - (nchunks - nalt)
                            ialt = CIN2 - ICH * nalt
                            rc = rp.tile([P, cols], dt.bfloat16, tag="rc")
                            nc.scalar.activation(
                                out=rc[:], in_=ppc[:],
                                func=mybir.ActivationFunctionType.Relu)
                            r3 = rc[:].rearrange("p (o i) -> p o i", o=OBL)
                            qc = qp.tile([P, cols], dt.bfloat16, tag="qa")
                            q3 = qc[:].rearrange("p (o i) -> p o i", o=OBL)
                            nc.gpsimd.tensor_tensor(
                                out=q3, in0=r3,
                                in1=xg[:, ialt:].unsqueeze(1).broadcast_to(
                                    [P, OBL, CIN2 - ialt]),
                                op=mybir.AluOpType.mult)
                            nc.vector.tensor_reduce(
                                out=msum[:, cb * OBL:(cb + 1) * OBL], in_=q3,
                                axis=mybir.AxisListType.X, op=mybir.AluOpType.add)
                    if nalt:
                        nc.tensor.matmul(aggw[:], lhsT=oh[:], rhs=msum[:],
                                         start=False, stop=(t == T - 1),
                                         skip_group_check=True)
                # finalize window: add bias, write out
                if is_l1:
                    hw_ = outp.tile([P, COUT], dt.bfloat16, tag="h1w")
                    nc.vector.tensor_tensor(out=hw_[:], in0=aggw[:], in1=b1_s[:],
                                            op=mybir.AluOpType.add)
                    nc.sync.dma_start(hloc[w * P:(w + 1) * P, :], hw_[:])
                else:
                    ow = outp.tile([P, COUT], dt.float32, tag="outw")
                    nc.vector.tensor_tensor(out=ow[:], in0=aggw[:], in1=b2_s[:],
                                            op=mybir.AluOpType.add)
                    nc.sync.dma_start(out_d[w * P:(w + 1) * P, :], ow[:])

        roots1 = []
        for w in range(WPC):
            lhsTw = rootp.tile([CIN1, P], dt.bfloat16, tag=f"rootl1_{w}")
            nc.sync.dma_start(lhsTw[:], xT_d[:, w * P:(w + 1) * P])
            roots1.append(lhsTw)
        layer(CIN1, A1_s, True, roots1)
        # layer-2 root lhsT transposes depend only on the LOCAL h1 slice, so
        # they overlap the layer-1 tail and the AllGather
        roots2 = []
        for w in range(WPC):
            lhsTw = rootp.tile([CIN2, P], dt.bfloat16, tag=f"rootl2_{w}")
            nc.sync.dma_start_transpose(lhsTw[:], hloc[w * P:(w + 1) * P, :])
            roots2.append(lhsTw)
        nc.gpsimd.collective_compute(
            "AllGather", mybir.AluOpType.bypass,
            replica_groups=[list(range(NCORES))],
            ins=[hloc[:].opt()], outs=[hglob[:].opt()])
        layer(CIN2, A2_s, False, roots2)

    nc.compile()
    return nc


def _pack(edge_index):
    """Relabel nodes into 128 windows of 128 nodes / exactly U edges each.

    Returns (perm, U, order) where perm[orig_node] = new node id and
    order = edge permutation grouping edges by destination window, padded.
    """
    dst = np.asarray(edge_index[1], dtype=np.int64)
    deg = np.bincount(dst, minlength=N).astype(np.int64)
    # LPT greedy: descending degree, least-loaded window with free slots
    nodes = np.argsort(-deg, kind="stable")
    loads = np.zeros(WINDOWS, dtype=np.int64)
    slots = np.zeros(WINDOWS, dtype=np.int64)
    wof = np.empty(N, dtype=np.int64)  # window of node
    for v in nodes:
        open_w = np.flatnonzero(slots < P)
        w = open_w[np.argmin(loads[open_w])]
        wof[v] = w
        loads[w] += deg[v]
        slots[w] += 1
    # repair toward exact target load by swapping nodes between windows
    target = E // WINDOWS
    if loads.max() > target:
        by_wd = {}  # (window, degree) -> list of nodes
        for v in range(N):
            by_wd.setdefault((wof[v], deg[v]), []).append(v)
        for _ in range(100000):
            over = int(np.argmax(loads))
            under = int(np.argmin(loads))
            if loads[over] <= target:
                break
            delta = min(loads[over] - target, target - loads[under])
            # find a swap pair with degree difference = d, largest d first
            done = False
            for d in range(int(delta), 0, -1):
                for da in range(int(deg.max()), d - 1, -1):
                    la = by_wd.get((over, da))
                    lb = by_wd.get((under, da - d))
                    if la and lb:
                        a, b = la.pop(), lb.pop()
                        wof[a], wof[b] = under, over
                        by_wd.setdefault((under, da), []).append(a)
                        by_wd.setdefault((over, da - d), []).append(b)
                        loads[over] -= d
                        loads[under] += d
                        done = True
                        break
                if done:
                    break
            if not done:
                break
    U = int(np.ceil(loads.max() / P) * P)
    # perm: nodes sorted by window -> new ids
    new_order = np.argsort(wof * N + np.arange(N), kind="stable")
    perm = np.empty(N, dtype=np.int64)
    perm[new_order] = np.arange(N)
    # edge order: group by destination window, pad each window to U
    ew = wof[dst]
    eorder = np.argsort(ew, kind="stable")
    counts = np.bincount(ew, minlength=WINDOWS)
    padded = np.full(WINDOWS * U, -1, dtype=np.int64)
    pos = 0
    for w in range(WINDOWS):
        c = int(counts[w])
        padded[w * U:w * U + c] = eorder[pos:pos + c]
        pos += c
    return perm, U, padded


def kernel(x, edge_index, edge_attr, A1, b1, A2, b2, root1, bias1, root2, bias2):
    x = np.asarray(x, dtype=np.float32)
    edge_index = np.asarray(edge_index)
    edge_attr = np.asarray(edge_attr, dtype=np.float32)

    perm, U, padded = _pack(edge_index)
    key = U
    if key not in _cached:
        _cached[key] = _build_program(U)
    nc = _cached[key]

    src = np.asarray(edge_index[0], dtype=np.int64)
    dst = np.asarray(edge_index[1], dtype=np.int64)
    valid = padded >= 0
    pe = np.where(valid, padded, 0)
    # per padded-edge data
    a01 = edge_attr[pe]                      # [W*U, 2]
    aug = valid.astype(np.float32)
    attrT_all = np.stack([a01[:, 0] * aug, a01[:, 1] * aug, aug]).astype(BF16)  # [3, W*U]
    srcn_all = np.where(valid, perm[src[pe]], 0).astype(np.int32)
    dstn = perm[dst[pe]]
    wof_e = np.arange(WINDOWS).repeat(U)
    dstrel_all = np.where(valid, dstn - wof_e * P, 0).astype(np.float32)

    x_pi = np.empty_like(x)
    x_pi[perm] = x
    x_bf = x_pi.astype(BF16)
    xg1_all = x_bf[srcn_all]                 # host pre-gather for layer 1

    A1aug = np.concatenate([A1, b1[None, :]], axis=0).astype(BF16)
    A2aug = np.concatenate([A2, b2[None, :]], axis=0).astype(BF16)
    # alt-route chunks cb cover o in [16*cb, 16*cb+16), i in [IALT, 64),
    # laid out o-major / i-minor
    cols = ICH * COUT
    j = np.arange(NALT * cols)
    cb = j // cols
    rem = j % cols
    ol = rem // (CIN2 - IALT)
    i = IALT + rem % (CIN2 - IALT)
    src_cols = i * COUT + (OBL * cb + ol)
    A2perm = A2aug.copy()
    A2perm[:, (8 - NALT) * cols:] = A2aug[:, src_cols]
    iota_np = np.broadcast_to(np.arange(P, dtype=np.float32), (P, P)).astype(BF16)
    b1_bc = np.broadcast_to(bias1, (P, COUT)).astype(np.float32).copy()
    b2_bc = np.broadcast_to(bias2, (P, COUT)).astype(np.float32).copy()
    shared = {
        "A1aug": A1aug, "A2aug": A2perm,
        "root1": np.asarray(root1.astype(BF16)),
        "root2": np.asarray(root2.astype(BF16)),
        "bias1": b1_bc, "bias2": b2_bc,
        "iota": np.asarray(iota_np),
    }
    EPC = WPC * U
    in_maps = []
    for c in range(NCORES):
        s = c * EPC
        m = dict(shared)
        m["attrT"] = attrT_all[:, s:s + EPC].copy()
        m["srcidx"] = srcn_all[s:s + EPC].reshape(EPC, 1).copy()
        m["dstrel"] = dstrel_all[s:s + EPC].reshape(EPC, 1).copy()
        m["xg1"] = np.ascontiguousarray(xg1_all[s:s + EPC])
        m["xT"] = np.ascontiguousarray(x_bf[c * NPC:(c + 1) * NPC].T)
        in_maps.append(m)

    res = run_bass_kernel_spmd(nc, in_maps, list(range(NCORES)),
                               **kernel.run_kwargs)
    kernel.last_result = res
    out_pi = np.concatenate([res.results[c]["out"] for c in range(NCORES)], axis=0)
    return out_pi[perm]


kernel.run_kwargs = {}
kernel.last_result = None



# revision 18
# speedup vs baseline: 1.1497x; 1.1452x over previous
"""NNConv (gnn_message_passing) Bass kernel for 8 Trainium2 NeuronCores.

Strategy (edge-parallel, dst-sharded):
- Host relabels nodes with a permutation so that the 16384 nodes form 128
  "windows" of 128 nodes, each window receiving exactly 512 edges (by
  destination).  Core c owns windows [16c, 16c+16): 2048 nodes / 8192 edges.
  This makes the device program identical across cores (pure SPMD); all
  per-core variation lives in the input data.
- Per 128-edge tile, on device:
    P   = attr_aug^T @ Aaug            (PE matmul -> PSUM, f32)
    q   = relu(P) * x[src] broadcast   (one fused DVE scalar_tensor_tensor,
                                        bf16 out; x gathered by indirect DMA)
    agg += onehot(dst)^T @ q           (PE matmul; zero-step output AP sums
                                        the c_in axis into the same PSUM
                                        columns while contracting edges)
  The root term (x @ root) and a per-window one-hot matmul accumulate into
  the same PSUM bank; bias is added during the PSUM->SBUF copy.
- h1 is exchanged between layers with an AllGather (bf16).
"""

import numpy as np
import ml_dtypes
from contextlib import ExitStack

import concourse.bass as bass
import concourse.tile as tile
from concourse import bacc, mybir
from concourse.bass import IndirectOffsetOnAxis
from concourse.bass_utils import run_bass_kernel_spmd

dt = mybir.dt
BF16 = ml_dtypes.bfloat16

N = 16384
E = 65536
NCORES = 8
P = 128                 # partitions / edges per tile
WINDOWS = 128           # global 128-node windows
WPC = WINDOWS // NCORES  # 16 windows per core
NPC = N // NCORES        # 2048 nodes per core
COUT = 64
CIN1 = 8
CIN2 = 64
ICH = 8                  # c_in values per P-gen chunk (chunk = ICH*COUT cols)
NALT = 0                 # layer-2 chunks routed Act-relu/Pool-mult/DVE-reduce
IALT = CIN2 - ICH * NALT  # first i covered by the alt route (32)
OBL = (ICH * COUT) // (CIN2 - IALT) if NALT else 0  # o-values per alt chunk

_cached = {}


def _build_program(U):
    """Build the SPMD Bass program. U = edges per window (multiple of 128)."""
    T = U // P  # tiles per window
    EPC = WPC * U  # edges per core

    nc = bacc.Bacc("TRN2", target_bir_lowering=False, debug=False,
                   num_devices=NCORES)

    attrT_d = nc.dram_tensor("attrT", [3, EPC], dt.bfloat16, kind="ExternalInput").ap()
    src_d = nc.dram_tensor("srcidx", [EPC, 1], dt.int32, kind="ExternalInput").ap()
    dstrel_d = nc.dram_tensor("dstrel", [EPC, 1], dt.float32, kind="ExternalInput").ap()
    A1aug_d = nc.dram_tensor("A1aug", [3, CIN1 * COUT], dt.bfloat16, kind="ExternalInput").ap()
    A2aug_d = nc.dram_tensor("A2aug", [3, CIN2 * COUT], dt.bfloat16, kind="ExternalInput").ap()
    xg1_d = nc.dram_tensor("xg1", [EPC, CIN1], dt.bfloat16, kind="ExternalInput").ap()
    xT_d = nc.dram_tensor("xT", [CIN1, NPC], dt.bfloat16, kind="ExternalInput").ap()
    r1_d = nc.dram_tensor("root1", [CIN1, COUT], dt.bfloat16, kind="ExternalInput").ap()
    r2_d = nc.dram_tensor("root2", [CIN2, COUT], dt.bfloat16, kind="ExternalInput").ap()
    b1_d = nc.dram_tensor("bias1", [P, COUT], dt.float32, kind="ExternalInput").ap()
    b2_d = nc.dram_tensor("bias2", [P, COUT], dt.float32, kind="ExternalInput").ap()
    iota_d = nc.dram_tensor("iota", [P, P], dt.bfloat16, kind="ExternalInput").ap()
    out_d = nc.dram_tensor("out", [NPC, COUT], dt.float32, kind="ExternalOutput").ap()

    with tile.TileContext(nc) as tc, ExitStack() as ctx, \
            nc.allow_low_precision("bf16 msum; abs tolerance 2e-2"):
        consts = ctx.enter_context(tc.tile_pool(name="consts", bufs=1))
        meta = ctx.enter_context(tc.tile_pool(name="meta", bufs=8))
        xgp = ctx.enter_context(tc.tile_pool(name="xgp", bufs=6))
        ohp = ctx.enter_context(tc.tile_pool(name="ohp", bufs=6))
        qp = ctx.enter_context(tc.tile_pool(name="qp", bufs=6))
        rp = ctx.enter_context(tc.tile_pool(name="rp", bufs=4))
        msp = ctx.enter_context(tc.tile_pool(name="msp", bufs=4))
        rootp = ctx.enter_context(tc.tile_pool(name="rootp", bufs=3))
        hp = ctx.enter_context(tc.tile_pool(name="hp", bufs=1))
        outp = ctx.enter_context(tc.tile_pool(name="outp", bufs=3))
        pp = ctx.enter_context(tc.tile_pool(name="pp", bufs=3, space="PSUM"))
        aggp = ctx.enter_context(tc.tile_pool(name="aggp", bufs=2, space="PSUM"))
        dramp = ctx.enter_context(tc.tile_pool(name="dram", bufs=1, space="DRAM"))

        A1_s = consts.tile([3, CIN1 * COUT], dt.bfloat16)
        nc.sync.dma_start(A1_s[:], A1aug_d[:])
        A2_s = consts.tile([3, CIN2 * COUT], dt.bfloat16)
        nc.sync.dma_start(A2_s[:], A2aug_d[:])
        iota_s = consts.tile([P, P], dt.bfloat16)
        nc.sync.dma_start(iota_s[:], iota_d[:])
        r1_s = consts.tile([CIN1, COUT], dt.bfloat16)
        nc.sync.dma_start(r1_s[:], r1_d[:])
        r2_s = consts.tile([CIN2, COUT], dt.bfloat16)
        nc.sync.dma_start(r2_s[:], r2_d[:])
        b1_s = consts.tile([P, COUT], dt.float32)
        nc.sync.dma_start(b1_s[:], b1_d[:])
        b2_s = consts.tile([P, COUT], dt.float32)
        nc.sync.dma_start(b2_s[:], b2_d[:])

        # h1 slice (local) and allgathered h1 (global), bf16
        hloc = dramp.tile([NPC, COUT], dt.bfloat16)
        hglob = dramp.tile([N, COUT], dt.bfloat16)

        def layer(cin, A_s, is_l1):
            nchunks = cin // ICH
            for w in range(WPC):
                aggw = aggp.tile([P, COUT], dt.float32, tag="aggw")
                # root-term matmul opens the accumulation (start=True)
                if is_l1:
                    lhsTw = rootp.tile([CIN1, P], dt.bfloat16, tag="rootl1")
                    nc.sync.dma_start(lhsTw[:], xT_d[:, w * P:(w + 1) * P])
                    nc.tensor.matmul(aggw[:], lhsT=lhsTw[:], rhs=r1_s[:],
                                     start=True, stop=False)
                else:
                    lhsTw = rootp.tile([CIN2, P], dt.bfloat16, tag="rootl2")
                    nc.sync.dma_start_transpose(
                        lhsTw[:], hloc[w * P:(w + 1) * P, :])
                    nc.tensor.matmul(aggw[:], lhsT=lhsTw[:], rhs=r2_s[:],
                                     start=True, stop=False)
                for t in range(T):
                    e0 = (w * T + t) * P
                    attr_t = meta.tile([3, P], dt.bfloat16, tag="attr")
                    nc.sync.dma_start(attr_t[:], attrT_d[:, e0:e0 + P])
                    dstt = meta.tile([P, 1], dt.float32, tag="dst")
                    nc.sync.dma_start(dstt[:], dstrel_d[e0:e0 + P, :])

                    cin_t = CIN1 if is_l1 else CIN2
                    xg = xgp.tile([P, cin_t], dt.bfloat16,
                                  tag="xg1" if is_l1 else "xg2")
                    if is_l1:
                        nc.sync.dma_start(xg[:], xg1_d[e0:e0 + P, :])
                    else:
                        srct = meta.tile([P, 1], dt.int32, tag="src")
                        nc.sync.dma_start(srct[:], src_d[e0:e0 + P, :])
                        nc.gpsimd.indirect_dma_start(
                            out=xg[:], out_offset=None, in_=hglob[:],
                            in_offset=IndirectOffsetOnAxis(ap=srct[:, :1], axis=0))

                    oh = ohp.tile([P, P], dt.bfloat16, tag="oh")
                    nc.gpsimd.tensor_scalar(
                        out=oh[:], in0=iota_s[:], scalar1=dstt[:, :1],
                        scalar2=None, op0=mybir.AluOpType.is_equal)

                    # layer 2: chunks 4-7 take the Act/Pool/DVE-reduce
                    # route ((o,i)-permuted A columns, i in [32,64))
                    nalt = 0 if is_l1 else NALT
                    if nalt:
                        msum = msp.tile([P, COUT], dt.bfloat16, tag="ms")
                    ich = min(cin, 2 * ICH)  # 8 (l1) or 16 (l2)
                    ngrp = cin // ich
                    for c in range(ngrp):
                        cols = ich * COUT
                        ppc = pp.tile([P, 2 * ICH * COUT], dt.float32,
                                      tag="ppc")
                        for h in range(0, cols, 512):
                            nc.tensor.matmul(
                                ppc[:, h:h + 512], lhsT=attr_t[:],
                                rhs=A_s[:, c * cols + h:c * cols + h + 512],
                                start=True, stop=True)
                        if True:
                            qc = qp.tile([P, cols], dt.bfloat16,
                                         tag="qc1" if is_l1 else "qc2")
                            q3 = qc[:].rearrange("p (i o) -> p i o", i=ich)
                            nc.vector.scalar_tensor_tensor(
                                out=q3, in0=ppc[:, :cols].rearrange("p (i o) -> p i o", i=ich),
                                scalar=0.0,
                                in1=xg[:, c * ich:(c + 1) * ich].to_broadcast(
                                    [P, ich, COUT]),
                                op0=mybir.AluOpType.max, op1=mybir.AluOpType.mult)
                            for h in range(0, cols, 512):
                                ii = 512 // COUT
                                nc.tensor.matmul(
                                    aggw[:].unsqueeze(1).broadcast_to([P, ii, COUT]),
                                    lhsT=oh[:],
                                    rhs=qc[:, h:h + 512].rearrange(
                                        "p (i o) -> p i o", i=ii),
                                    start=False,
                                    stop=(t == T - 1 and c == ngrp - 1
                                          and h + 512 >= cols and nalt == 0),
                                    skip_group_check=True)
                        else:
                            # (o,i) chunk: o-block of OBL, i in [IALT, 64)
                            cb = c - (nchunks - nalt)
                            ialt = CIN2 - ICH * nalt
                            rc = rp.tile([P, cols], dt.bfloat16, tag="rc")
                            nc.scalar.activation(
                                out=rc[:], in_=ppc[:],
                                func=mybir.ActivationFunctionType.Relu)
                            r3 = rc[:].rearrange("p (o i) -> p o i", o=OBL)
                            qc = qp.tile([P, cols], dt.bfloat16, tag="qa")
                            q3 = qc[:].rearrange("p (o i) -> p o i", o=OBL)
                            nc.gpsimd.tensor_tensor(
                                out=q3, in0=r3,
                                in1=xg[:, ialt:].unsqueeze(1).broadcast_to(
                                    [P, OBL, CIN2 - ialt]),
                                op=mybir.AluOpType.mult)
                            nc.vector.tensor_reduce(
                                out=msum[:, cb * OBL:(cb + 1) * OBL], in_=q3,
                                axis=mybir.AxisListType.X, op=mybir.AluOpType.add)
                    if nalt:
                        nc.tensor.matmul(aggw[:], lhsT=oh[:], rhs=msum[:],
                                         start=False, stop=(t == T - 1),
                                         skip_group_check=True)
                # finalize window: add bias, write out
                if is_l1:
                    hw_ = outp.tile([P, COUT], dt.bfloat16, tag="h1w")
                    nc.vector.tensor_tensor(out=hw_[:], in0=aggw[:], in1=b1_s[:],
                                            op=mybir.AluOpType.add)
                    nc.sync.dma_start(hloc[w * P:(w + 1) * P, :], hw_[:])
                else:
                    ow = outp.tile([P, COUT], dt.float32, tag="outw")
                    nc.vector.tensor_tensor(out=ow[:], in0=aggw[:], in1=b2_s[:],
                                            op=mybir.AluOpType.add)
                    nc.sync.dma_start(out_d[w * P:(w + 1) * P, :], ow[:])

        layer(CIN1, A1_s, True)
        nc.gpsimd.collective_compute(
            "AllGather", mybir.AluOpType.bypass,
            replica_groups=[list(range(NCORES))],
            ins=[hloc[:].opt()], outs=[hglob[:].opt()])
        layer(CIN2, A2_s, False)

    nc.compile()
    return nc


def _pack(edge_index):
    """Relabel nodes into 128 windows of 128 nodes / exactly U edges each.

    Returns (perm, U, order) where perm[orig_node] = new node id and
    order = edge permutation grouping edges by destination window, padded.
    """
    dst = np.asarray(edge_index[1], dtype=np.int64)
    deg = np.bincount(dst, minlength=N).astype(np.int64)
    # LPT greedy: descending degree, least-loaded window with free slots
    nodes = np.argsort(-deg, kind="stable")
    loads = np.zeros(WINDOWS, dtype=np.int64)
    slots = np.zeros(WINDOWS, dtype=np.int64)
    wof = np.empty(N, dtype=np.int64)  # window of node
    for v in nodes:
        open_w = np.flatnonzero(slots < P)
        w = open_w[np.argmin(loads[open_w])]
        wof[v] = w
        loads[w] += deg[v]
        slots[w] += 1
    # repair toward exact target load by swapping nodes between windows
    target = E // WINDOWS
    if loads.max() > target:
        by_wd = {}  # (window, degree) -> list of nodes
        for v in range(N):
            by_wd.setdefault((wof[v], deg[v]), []).append(v)
        for _ in range(100000):
            over = int(np.argmax(loads))
            under = int(np.argmin(loads))
            if loads[over] <= target:
                break
            delta = min(loads[over] - target, target - loads[under])
            # find a swap pair with degree difference = d, largest d first
            done = False
            for d in range(int(delta), 0, -1):
                for da in range(int(deg.max()), d - 1, -1):
                    la = by_wd.get((over, da))
                    lb = by_wd.get((under, da - d))
                    if la and lb:
                        a, b = la.pop(), lb.pop()
                        wof[a], wof[b] = under, over
                        by_wd.setdefault((under, da), []).append(a)
                        by_wd.setdefault((over, da - d), []).append(b)
                        loads[over] -= d
                        loads[under] += d
                        done = True
                        break
                if done:
                    break
            if not done:
                break
    U = int(np.ceil(loads.max() / P) * P)
    # perm: nodes sorted by window -> new ids
    new_order = np.argsort(wof * N + np.arange(N), kind="stable")
    perm = np.empty(N, dtype=np.int64)
    perm[new_order] = np.arange(N)
    # edge order: group by destination window, pad each window to U
    ew = wof[dst]
    eorder = np.argsort(ew, kind="stable")
    counts = np.bincount(ew, minlength=WINDOWS)
    padded = np.full(WINDOWS * U, -1, dtype=np.int64)
    pos = 0
    for w in range(WINDOWS):
        c = int(counts[w])
        padded[w * U:w * U + c] = eorder[pos:pos + c]
        pos += c
    return perm, U, padded


def kernel(x, edge_index, edge_attr, A1, b1, A2, b2, root1, bias1, root2, bias2):
    x = np.asarray(x, dtype=np.float32)
    edge_index = np.asarray(edge_index)
    edge_attr = np.asarray(edge_attr, dtype=np.float32)

    perm, U, padded = _pack(edge_index)
    key = U
    if key not in _cached:
        _cached[key] = _build_program(U)
    nc = _cached[key]

    src = np.asarray(edge_index[0], dtype=np.int64)
    dst = np.asarray(edge_index[1], dtype=np.int64)
    valid = padded >= 0
    pe = np.where(valid, padded, 0)
    # per padded-edge data
    a01 = edge_attr[pe]                      # [W*U, 2]
    aug = valid.astype(np.float32)
    attrT_all = np.stack([a01[:, 0] * aug, a01[:, 1] * aug, aug]).astype(BF16)  # [3, W*U]
    srcn_all = np.where(valid, perm[src[pe]], 0).astype(np.int32)
    dstn = perm[dst[pe]]
    wof_e = np.arange(WINDOWS).repeat(U)
    dstrel_all = np.where(valid, dstn - wof_e * P, 0).astype(np.float32)

    x_pi = np.empty_like(x)
    x_pi[perm] = x
    x_bf = x_pi.astype(BF16)
    xg1_all = x_bf[srcn_all]                 # host pre-gather for layer 1

    A1aug = np.concatenate([A1, b1[None, :]], axis=0).astype(BF16)
    A2aug = np.concatenate([A2, b2[None, :]], axis=0).astype(BF16)
    # alt-route chunks cb cover o in [16*cb, 16*cb+16), i in [IALT, 64),
    # laid out o-major / i-minor
    cols = ICH * COUT
    j = np.arange(NALT * cols)
    cb = j // cols
    rem = j % cols
    ol = rem // (CIN2 - IALT)
    i = IALT + rem % (CIN2 - IALT)
    src_cols = i * COUT + (OBL * cb + ol)
    A2perm = A2aug.copy()
    A2perm[:, (8 - NALT) * cols:] = A2aug[:, src_cols]
    iota_np = np.broadcast_to(np.arange(P, dtype=np.float32), (P, P)).astype(BF16)
    b1_bc = np.broadcast_to(bias1, (P, COUT)).astype(np.float32).copy()
    b2_bc = np.broadcast_to(bias2, (P, COUT)).astype(np.float32).copy()
    shared = {
        "A1aug": A1aug, "A2aug": A2perm,
        "root1": np.asarray(root1.astype(BF16)),
        "root2": np.asarray(root2.astype(BF16)),
        "bias1": b1_bc, "bias2": b2_bc,
        "iota": np.asarray(iota_np),
    }
    EPC = WPC * U
    in_maps = []
    for c in range(NCORES):
        s = c * EPC
        m = dict(shared)
        m["attrT"] = attrT_all[:, s:s + EPC].copy()
        m["srcidx"] = srcn_all[s:s + EPC].reshape(EPC, 1).copy()
        m["dstrel"] = dstrel_all[s:s + EPC].reshape(EPC, 1).copy()
        m["xg1"] = np.ascontiguousarray(xg1_all[s:s + EPC])
        m["xT"] = np.ascontiguousarray(x_bf[c * NPC:(c + 1) * NPC].T)
        in_maps.append(m)

    res = run_bass_kernel_spmd(nc, in_maps, list(range(NCORES)),
                               **kernel.run_kwargs)
    kernel.last_result = res
    out_pi = np.concatenate([res.results[c]["out"] for c in range(NCORES)], axis=0)
    return out_pi[perm]


kernel.run_kwargs = {}
kernel.last_result = None



# revision 20
# speedup vs baseline: 1.1999x; 1.0437x over previous
"""NNConv (gnn_message_passing) Bass kernel for 8 Trainium2 NeuronCores.

Strategy (edge-parallel, dst-sharded):
- Host relabels nodes with a permutation so that the 16384 nodes form 128
  "windows" of 128 nodes, each window receiving exactly 512 edges (by
  destination).  Core c owns windows [16c, 16c+16): 2048 nodes / 8192 edges.
  This makes the device program identical across cores (pure SPMD); all
  per-core variation lives in the input data.
- Per 128-edge tile, on device:
    P   = attr_aug^T @ Aaug            (PE matmul -> PSUM, f32)
    q   = relu(P) * x[src] broadcast   (one fused DVE scalar_tensor_tensor,
                                        bf16 out; x gathered by indirect DMA)
    agg += onehot(dst)^T @ q           (PE matmul; zero-step output AP sums
                                        the c_in axis into the same PSUM
                                        columns while contracting edges)
  The root term (x @ root) and a per-window one-hot matmul accumulate into
  the same PSUM bank; bias is added during the PSUM->SBUF copy.
- h1 is exchanged between layers with an AllGather (bf16).
"""

import numpy as np
import ml_dtypes
from contextlib import ExitStack

import concourse.bass as bass
import concourse.tile as tile
from concourse import bacc, mybir
from concourse.bass import IndirectOffsetOnAxis
from concourse.bass_utils import run_bass_kernel_spmd

dt = mybir.dt
BF16 = ml_dtypes.bfloat16

N = 16384
E = 65536
NCORES = 8
P = 128                 # partitions / edges per tile
WINDOWS = 128           # global 128-node windows
WPC = WINDOWS // NCORES  # 16 windows per core
NPC = N // NCORES        # 2048 nodes per core
COUT = 64
CIN1 = 8
CIN2 = 64
ICH = 8                  # c_in values per P-gen chunk (chunk = ICH*COUT cols)
NALT = 0                 # layer-2 chunks routed Act-relu/Pool-mult/DVE-reduce
IALT = CIN2 - ICH * NALT  # first i covered by the alt route (32)
OBL = (ICH * COUT) // (CIN2 - IALT) if NALT else 0  # o-values per alt chunk

_cached = {}


def _build_program(U):
    """Build the SPMD Bass program. U = edges per window (multiple of 128)."""
    T = U // P  # tiles per window
    EPC = WPC * U  # edges per core

    nc = bacc.Bacc("TRN2", target_bir_lowering=False, debug=False,
                   num_devices=NCORES)

    attrT_d = nc.dram_tensor("attrT", [3, EPC], dt.bfloat16, kind="ExternalInput").ap()
    src_d = nc.dram_tensor("srcidx", [EPC, 1], dt.int32, kind="ExternalInput").ap()
    dstrel_d = nc.dram_tensor("dstrel", [EPC, 1], dt.float32, kind="ExternalInput").ap()
    A1aug_d = nc.dram_tensor("A1aug", [3, CIN1 * COUT], dt.bfloat16, kind="ExternalInput").ap()
    A2aug_d = nc.dram_tensor("A2aug", [3, CIN2 * COUT], dt.bfloat16, kind="ExternalInput").ap()
    xg1_d = nc.dram_tensor("xg1", [EPC, CIN1], dt.bfloat16, kind="ExternalInput").ap()
    xT_d = nc.dram_tensor("xT", [CIN1, NPC], dt.bfloat16, kind="ExternalInput").ap()
    r1_d = nc.dram_tensor("root1", [CIN1, COUT], dt.bfloat16, kind="ExternalInput").ap()
    r2_d = nc.dram_tensor("root2", [CIN2, COUT], dt.bfloat16, kind="ExternalInput").ap()
    b1_d = nc.dram_tensor("bias1", [P, COUT], dt.float32, kind="ExternalInput").ap()
    b2_d = nc.dram_tensor("bias2", [P, COUT], dt.float32, kind="ExternalInput").ap()
    iota_d = nc.dram_tensor("iota", [P, P], dt.bfloat16, kind="ExternalInput").ap()
    out_d = nc.dram_tensor("out", [NPC, COUT], dt.float32, kind="ExternalOutput").ap()

    with tile.TileContext(nc) as tc, ExitStack() as ctx, \
            nc.allow_low_precision("bf16 msum; abs tolerance 2e-2"):
        consts = ctx.enter_context(tc.tile_pool(name="consts", bufs=1))
        meta = ctx.enter_context(tc.tile_pool(name="meta", bufs=8))
        xgp = ctx.enter_context(tc.tile_pool(name="xgp", bufs=6))
        ohp = ctx.enter_context(tc.tile_pool(name="ohp", bufs=6))
        qp = ctx.enter_context(tc.tile_pool(name="qp", bufs=6))
        rp = ctx.enter_context(tc.tile_pool(name="rp", bufs=4))
        msp = ctx.enter_context(tc.tile_pool(name="msp", bufs=4))
        rootp = ctx.enter_context(tc.tile_pool(name="rootp", bufs=3))
        hp = ctx.enter_context(tc.tile_pool(name="hp", bufs=1))
        outp = ctx.enter_context(tc.tile_pool(name="outp", bufs=3))
        pp = ctx.enter_context(tc.tile_pool(name="pp", bufs=3, space="PSUM"))
        aggp = ctx.enter_context(tc.tile_pool(name="aggp", bufs=2, space="PSUM"))
        dramp = ctx.enter_context(tc.tile_pool(name="dram", bufs=1, space="DRAM"))

        A1_s = consts.tile([3, CIN1 * COUT], dt.bfloat16)
        nc.sync.dma_start(A1_s[:], A1aug_d[:])
        A2_s = consts.tile([3, CIN2 * COUT], dt.bfloat16)
        nc.sync.dma_start(A2_s[:], A2aug_d[:])
        iota_s = consts.tile([P, P], dt.bfloat16)
        nc.sync.dma_start(iota_s[:], iota_d[:])
        r1_s = consts.tile([CIN1, COUT], dt.bfloat16)
        nc.sync.dma_start(r1_s[:], r1_d[:])
        r2_s = consts.tile([CIN2, COUT], dt.bfloat16)
        nc.sync.dma_start(r2_s[:], r2_d[:])
        b1_s = consts.tile([P, COUT], dt.float32)
        nc.sync.dma_start(b1_s[:], b1_d[:])
        b2_s = consts.tile([P, COUT], dt.float32)
        nc.sync.dma_start(b2_s[:], b2_d[:])

        # h1 slice (local) and allgathered h1 (global), bf16
        hloc = dramp.tile([NPC, COUT], dt.bfloat16)
        hglob = dramp.tile([N, COUT], dt.bfloat16)

        def layer(cin, A_s, is_l1):
            nchunks = cin // ICH
            for w in range(WPC):
                aggw = aggp.tile([P, COUT], dt.float32, tag="aggw")
                # root-term matmul opens the accumulation (start=True)
                if is_l1:
                    lhsTw = rootp.tile([CIN1, P], dt.bfloat16, tag="rootl1")
                    nc.sync.dma_start(lhsTw[:], xT_d[:, w * P:(w + 1) * P])
                    nc.tensor.matmul(aggw[:], lhsT=lhsTw[:], rhs=r1_s[:],
                                     start=True, stop=False)
                else:
                    lhsTw = rootp.tile([CIN2, P], dt.bfloat16, tag="rootl2")
                    nc.sync.dma_start_transpose(
                        lhsTw[:], hloc[w * P:(w + 1) * P, :])
                    nc.tensor.matmul(aggw[:], lhsT=lhsTw[:], rhs=r2_s[:],
                                     start=True, stop=False)
                for t in range(T):
                    e0 = (w * T + t) * P
                    attr_t = meta.tile([3, P], dt.bfloat16, tag="attr")
                    nc.sync.dma_start(attr_t[:], attrT_d[:, e0:e0 + P])
                    dstt = meta.tile([P, 1], dt.float32, tag="dst")
                    nc.sync.dma_start(dstt[:], dstrel_d[e0:e0 + P, :])

                    cin_t = CIN1 if is_l1 else CIN2
                    xg = xgp.tile([P, cin_t], dt.bfloat16,
                                  tag="xg1" if is_l1 else "xg2")
                    if is_l1:
                        nc.sync.dma_start(xg[:], xg1_d[e0:e0 + P, :])
                    else:
                        srct = meta.tile([P, 1], dt.int32, tag="src")
                        nc.sync.dma_start(srct[:], src_d[e0:e0 + P, :])
                        nc.gpsimd.indirect_dma_start(
                            out=xg[:], out_offset=None, in_=hglob[:],
                            in_offset=IndirectOffsetOnAxis(ap=srct[:, :1], axis=0))

                    oh = ohp.tile([P, P], dt.bfloat16, tag="oh")
                    (nc.vector if is_l1 else nc.gpsimd).tensor_scalar(
                        out=oh[:], in0=iota_s[:], scalar1=dstt[:, :1],
                        scalar2=None, op0=mybir.AluOpType.is_equal)

                    # layer 2: chunks 4-7 take the Act/Pool/DVE-reduce
                    # route ((o,i)-permuted A columns, i in [32,64))
                    nalt = 0 if is_l1 else NALT
                    if nalt:
                        msum = msp.tile([P, COUT], dt.bfloat16, tag="ms")
                    ich = min(cin, 2 * ICH)  # 8 (l1) or 16 (l2)
                    ngrp = cin // ich
                    for c in range(ngrp):
                        cols = ich * COUT
                        ppc = pp.tile([P, 2 * ICH * COUT], dt.float32,
                                      tag="ppc")
                        for h in range(0, cols, 512):
                            nc.tensor.matmul(
                                ppc[:, h:h + 512], lhsT=attr_t[:],
                                rhs=A_s[:, c * cols + h:c * cols + h + 512],
                                start=True, stop=True)
                        if True:
                            qc = qp.tile([P, cols], dt.bfloat16,
                                         tag="qc1" if is_l1 else "qc2")
                            q3 = qc[:].rearrange("p (i o) -> p i o", i=ich)
                            nc.vector.scalar_tensor_tensor(
                                out=q3, in0=ppc[:, :cols].rearrange("p (i o) -> p i o", i=ich),
                                scalar=0.0,
                                in1=xg[:, c * ich:(c + 1) * ich].to_broadcast(
                                    [P, ich, COUT]),
                                op0=mybir.AluOpType.max, op1=mybir.AluOpType.mult)
                            for h in range(0, cols, 512):
                                ii = 512 // COUT
                                nc.tensor.matmul(
                                    aggw[:].unsqueeze(1).broadcast_to([P, ii, COUT]),
                                    lhsT=oh[:],
                                    rhs=qc[:, h:h + 512].rearrange(
                                        "p (i o) -> p i o", i=ii),
                                    start=False,
                                    stop=(t == T - 1 and c == ngrp - 1
                                          and h + 512 >= cols and nalt == 0),
                                    skip_group_check=True)
                        else:
                            # (o,i) chunk: o-block of OBL, i in [IALT, 64)
                            cb = c - (nchunks - nalt)
                            ialt = CIN2 - ICH * nalt
                            rc = rp.tile([P, cols], dt.bfloat16, tag="rc")
                            nc.scalar.activation(
                                out=rc[:], in_=ppc[:],
                                func=mybir.ActivationFunctionType.Relu)
                            r3 = rc[:].rearrange("p (o i) -> p o i", o=OBL)
                            qc = qp.tile([P, cols], dt.bfloat16, tag="qa")
                            q3 = qc[:].rearrange("p (o i) -> p o i", o=OBL)
                            nc.gpsimd.tensor_tensor(
                                out=q3, in0=r3,
                                in1=xg[:, ialt:].unsqueeze(1).broadcast_to(
                                    [P, OBL, CIN2 - ialt]),
                                op=mybir.AluOpType.mult)
                            nc.vector.tensor_reduce(
                                out=msum[:, cb * OBL:(cb + 1) * OBL], in_=q3,
                                axis=mybir.AxisListType.X, op=mybir.AluOpType.add)
                    if nalt:
                        nc.tensor.matmul(aggw[:], lhsT=oh[:], rhs=msum[:],
                                         start=False, stop=(t == T - 1),
                                         skip_group_check=True)
                # finalize window: add bias, write out
                if is_l1:
                    hw_ = outp.tile([P, COUT], dt.bfloat16, tag="h1w")
                    nc.vector.tensor_tensor(out=hw_[:], in0=aggw[:], in1=b1_s[:],
                                            op=mybir.AluOpType.add)
                    nc.sync.dma_start(hloc[w * P:(w + 1) * P, :], hw_[:])
                else:
                    ow = outp.tile([P, COUT], dt.float32, tag="outw")
                    nc.vector.tensor_tensor(out=ow[:], in0=aggw[:], in1=b2_s[:],
                                            op=mybir.AluOpType.add)
                    nc.sync.dma_start(out_d[w * P:(w + 1) * P, :], ow[:])

        layer(CIN1, A1_s, True)
        # quarter-AllGathers with contiguous outputs: quarter q of every
        # core's hloc lands at hglob[q*8*qn : (q+1)*8*qn] (core-major inside);
        # the l2 gather indices are remapped host-side to this layout
        CQ = 4
        qn = NPC // CQ
        for q in range(CQ):
            nc.gpsimd.collective_compute(
                "AllGather", mybir.AluOpType.bypass,
                replica_groups=[list(range(NCORES))],
                ins=[hloc[q * qn:(q + 1) * qn, :].opt()],
                outs=[hglob[q * NCORES * qn:(q + 1) * NCORES * qn, :].opt()])
        layer(CIN2, A2_s, False)

    nc.compile()
    return nc


def _pack(edge_index):
    """Relabel nodes into 128 windows of 128 nodes / exactly U edges each.

    Returns (perm, U, order) where perm[orig_node] = new node id and
    order = edge permutation grouping edges by destination window, padded.
    """
    dst = np.asarray(edge_index[1], dtype=np.int64)
    deg = np.bincount(dst, minlength=N).astype(np.int64)
    # LPT greedy: descending degree, least-loaded window with free slots
    nodes = np.argsort(-deg, kind="stable")
    loads = np.zeros(WINDOWS, dtype=np.int64)
    slots = np.zeros(WINDOWS, dtype=np.int64)
    wof = np.empty(N, dtype=np.int64)  # window of node
    for v in nodes:
        open_w = np.flatnonzero(slots < P)
        w = open_w[np.argmin(loads[open_w])]
        wof[v] = w
        loads[w] += deg[v]
        slots[w] += 1
    # repair toward exact target load by swapping nodes between windows
    target = E // WINDOWS
    if loads.max() > target:
        by_wd = {}  # (window, degree) -> list of nodes
        for v in range(N):
            by_wd.setdefault((wof[v], deg[v]), []).append(v)
        for _ in range(100000):
            over = int(np.argmax(loads))
            under = int(np.argmin(loads))
            if loads[over] <= target:
                break
            delta = min(loads[over] - target, target - loads[under])
            # find a swap pair with degree difference = d, largest d first
            done = False
            for d in range(int(delta), 0, -1):
                for da in range(int(deg.max()), d - 1, -1):
                    la = by_wd.get((over, da))
                    lb = by_wd.get((under, da - d))
                    if la and lb:
                        a, b = la.pop(), lb.pop()
                        wof[a], wof[b] = under, over
                        by_wd.setdefault((under, da), []).append(a)
                        by_wd.setdefault((over, da - d), []).append(b)
                        loads[over] -= d
                        loads[under] += d
                        done = True
                        break
                if done:
                    break
            if not done:
                break
    U = int(np.ceil(loads.max() / P) * P)
    # perm: nodes sorted by window -> new ids
    new_order = np.argsort(wof * N + np.arange(N), kind="stable")
    perm = np.empty(N, dtype=np.int64)
    perm[new_order] = np.arange(N)
    # edge order: group by destination window, pad each window to U
    ew = wof[dst]
    eorder = np.argsort(ew, kind="stable")
    counts = np.bincount(ew, minlength=WINDOWS)
    padded = np.full(WINDOWS * U, -1, dtype=np.int64)
    pos = 0
    for w in range(WINDOWS):
        c = int(counts[w])
        padded[w * U:w * U + c] = eorder[pos:pos + c]
        pos += c
    return perm, U, padded


def kernel(x, edge_index, edge_attr, A1, b1, A2, b2, root1, bias1, root2, bias2):
    x = np.asarray(x, dtype=np.float32)
    edge_index = np.asarray(edge_index)
    edge_attr = np.asarray(edge_attr, dtype=np.float32)

    perm, U, padded = _pack(edge_index)
    key = U
    if key not in _cached:
        _cached[key] = _build_program(U)
    nc = _cached[key]

    src = np.asarray(edge_index[0], dtype=np.int64)
    dst = np.asarray(edge_index[1], dtype=np.int64)
    valid = padded >= 0
    pe = np.where(valid, padded, 0)
    # per padded-edge data
    a01 = edge_attr[pe]                      # [W*U, 2]
    aug = valid.astype(np.float32)
    attrT_all = np.stack([a01[:, 0] * aug, a01[:, 1] * aug, aug]).astype(BF16)  # [3, W*U]
    srcn_all = np.where(valid, perm[src[pe]], 0).astype(np.int32)
    dstn = perm[dst[pe]]
    wof_e = np.arange(WINDOWS).repeat(U)
    dstrel_all = np.where(valid, dstn - wof_e * P, 0).astype(np.float32)

    qn = NPC // 4
    sn64 = srcn_all.astype(np.int64)
    src2_all = ((sn64 % NPC) // qn * (NCORES * qn) + (sn64 // NPC) * qn
                + (sn64 % qn)).astype(np.int32)
    x_pi = np.empty_like(x)
    x_pi[perm] = x
    x_bf = x_pi.astype(BF16)
    xg1_all = x_bf[srcn_all]                 # host pre-gather for layer 1

    A1aug = np.concatenate([A1, b1[None, :]], axis=0).astype(BF16)
    A2aug = np.concatenate([A2, b2[None, :]], axis=0).astype(BF16)
    # alt-route chunks cb cover o in [16*cb, 16*cb+16), i in [IALT, 64),
    # laid out o-major / i-minor
    cols = ICH * COUT
    j = np.arange(NALT * cols)
    cb = j // cols
    rem = j % cols
    ol = rem // (CIN2 - IALT)
    i = IALT + rem % (CIN2 - IALT)
    src_cols = i * COUT + (OBL * cb + ol)
    A2perm = A2aug.copy()
    A2perm[:, (8 - NALT) * cols:] = A2aug[:, src_cols]
    iota_np = np.broadcast_to(np.arange(P, dtype=np.float32), (P, P)).astype(BF16)
    b1_bc = np.broadcast_to(bias1, (P, COUT)).astype(np.float32).copy()
    b2_bc = np.broadcast_to(bias2, (P, COUT)).astype(np.float32).copy()
    shared = {
        "A1aug": A1aug, "A2aug": A2perm,
        "root1": np.asarray(root1.astype(BF16)),
        "root2": np.asarray(root2.astype(BF16)),
        "bias1": b1_bc, "bias2": b2_bc,
        "iota": np.asarray(iota_np),
    }
    EPC = WPC * U
    in_maps = []
    for c in range(NCORES):
        s = c * EPC
        m = dict(shared)
        m["attrT"] = attrT_all[:, s:s + EPC].copy()
        m["srcidx"] = src2_all[s:s + EPC].reshape(EPC, 1).copy()
        m["dstrel"] = dstrel_all[s:s + EPC].reshape(EPC, 1).copy()
        m["xg1"] = np.ascontiguousarray(xg1_all[s:s + EPC])
        m["xT"] = np.ascontiguousarray(x_bf[c * NPC:(c + 1) * NPC].T)
        in_maps.append(m)

    res = run_bass_kernel_spmd(nc, in_maps, list(range(NCORES)),
                               **kernel.run_kwargs)
    kernel.last_result = res
    out_pi = np.concatenate([res.results[c]["out"] for c in range(NCORES)], axis=0)
    return out_pi[perm]


kernel.run_kwargs = {}
kernel.last_result = None



# revision 21
# speedup vs baseline: 1.2524x; 1.0437x over previous
"""NNConv (gnn_message_passing) Bass kernel for 8 Trainium2 NeuronCores.

Strategy (edge-parallel, dst-sharded):
- Host relabels nodes with a permutation so that the 16384 nodes form 128
  "windows" of 128 nodes, each window receiving exactly 512 edges (by
  destination).  Core c owns windows [16c, 16c+16): 2048 nodes / 8192 edges.
  This makes the device program identical across cores (pure SPMD); all
  per-core variation lives in the input data.
- Per 128-edge tile, on device:
    P   = attr_aug^T @ Aaug            (PE matmul -> PSUM, f32)
    q   = relu(P) * x[src] broadcast   (one fused DVE scalar_tensor_tensor,
                                        bf16 out; x gathered by indirect DMA)
    agg += onehot(dst)^T @ q           (PE matmul; zero-step output AP sums
                                        the c_in axis into the same PSUM
                                        columns while contracting edges)
  The root term (x @ root) and a per-window one-hot matmul accumulate into
  the same PSUM bank; bias is added during the PSUM->SBUF copy.
- h1 is exchanged between layers with an AllGather (bf16).
"""

import numpy as np
import ml_dtypes
from contextlib import ExitStack

import concourse.bass as bass
import concourse.tile as tile
from concourse import bacc, mybir
from concourse.bass import IndirectOffsetOnAxis
from concourse.bass_utils import run_bass_kernel_spmd

dt = mybir.dt
BF16 = ml_dtypes.bfloat16

N = 16384
E = 65536
NCORES = 8
P = 128                 # partitions / edges per tile
WINDOWS = 128           # global 128-node windows
WPC = WINDOWS // NCORES  # 16 windows per core
NPC = N // NCORES        # 2048 nodes per core
COUT = 64
CIN1 = 8
CIN2 = 64
ICH = 8                  # c_in values per P-gen chunk (chunk = ICH*COUT cols)
NALT = 0                 # layer-2 chunks routed Act-relu/Pool-mult/DVE-reduce
IALT = CIN2 - ICH * NALT  # first i covered by the alt route (32)
OBL = (ICH * COUT) // (CIN2 - IALT) if NALT else 0  # o-values per alt chunk

_cached = {}


def _build_program(U):
    """Build the SPMD Bass program. U = edges per window (multiple of 128)."""
    T = U // P  # tiles per window
    EPC = WPC * U  # edges per core

    nc = bacc.Bacc("TRN2", target_bir_lowering=False, debug=False,
                   num_devices=NCORES)

    attrT_d = nc.dram_tensor("attrT", [3, EPC], dt.bfloat16, kind="ExternalInput").ap()
    src_d = nc.dram_tensor("srcidx", [EPC, 1], dt.int32, kind="ExternalInput").ap()
    dstrel_d = nc.dram_tensor("dstrel", [EPC, 1], dt.float32, kind="ExternalInput").ap()
    A1aug_d = nc.dram_tensor("A1aug", [3, CIN1 * COUT], dt.bfloat16, kind="ExternalInput").ap()
    A2aug_d = nc.dram_tensor("A2aug", [3, CIN2 * COUT], dt.bfloat16, kind="ExternalInput").ap()
    xg1_d = nc.dram_tensor("xg1", [EPC, CIN1], dt.bfloat16, kind="ExternalInput").ap()
    xT_d = nc.dram_tensor("xT", [CIN1, NPC], dt.bfloat16, kind="ExternalInput").ap()
    r1_d = nc.dram_tensor("root1", [CIN1, COUT], dt.bfloat16, kind="ExternalInput").ap()
    r2_d = nc.dram_tensor("root2", [CIN2, COUT], dt.bfloat16, kind="ExternalInput").ap()
    b1_d = nc.dram_tensor("bias1", [P, COUT], dt.float32, kind="ExternalInput").ap()
    b2_d = nc.dram_tensor("bias2", [P, COUT], dt.float32, kind="ExternalInput").ap()
    iota_d = nc.dram_tensor("iota", [P, P], dt.bfloat16, kind="ExternalInput").ap()
    out_d = nc.dram_tensor("out", [NPC, COUT], dt.float32, kind="ExternalOutput").ap()

    with tile.TileContext(nc) as tc, ExitStack() as ctx, \
            nc.allow_low_precision("bf16 msum; abs tolerance 2e-2"):
        consts = ctx.enter_context(tc.tile_pool(name="consts", bufs=1))
        meta = ctx.enter_context(tc.tile_pool(name="meta", bufs=8))
        xgp = ctx.enter_context(tc.tile_pool(name="xgp", bufs=6))
        ohp = ctx.enter_context(tc.tile_pool(name="ohp", bufs=6))
        qp = ctx.enter_context(tc.tile_pool(name="qp", bufs=6))
        rp = ctx.enter_context(tc.tile_pool(name="rp", bufs=4))
        msp = ctx.enter_context(tc.tile_pool(name="msp", bufs=4))
        rootp = ctx.enter_context(tc.tile_pool(name="rootp", bufs=3))
        hp = ctx.enter_context(tc.tile_pool(name="hp", bufs=1))
        outp = ctx.enter_context(tc.tile_pool(name="outp", bufs=3))
        pp = ctx.enter_context(tc.tile_pool(name="pp", bufs=3, space="PSUM"))
        aggp = ctx.enter_context(tc.tile_pool(name="aggp", bufs=2, space="PSUM"))
        dramp = ctx.enter_context(tc.tile_pool(name="dram", bufs=1, space="DRAM"))

        A1_s = consts.tile([3, CIN1 * COUT], dt.bfloat16)
        nc.sync.dma_start(A1_s[:], A1aug_d[:])
        A2_s = consts.tile([3, CIN2 * COUT], dt.bfloat16)
        nc.sync.dma_start(A2_s[:], A2aug_d[:])
        iota_s = consts.tile([P, P], dt.bfloat16)
        nc.sync.dma_start(iota_s[:], iota_d[:])
        r1_s = consts.tile([CIN1, COUT], dt.bfloat16)
        nc.sync.dma_start(r1_s[:], r1_d[:])
        r2_s = consts.tile([CIN2, COUT], dt.bfloat16)
        nc.sync.dma_start(r2_s[:], r2_d[:])
        b1_s = consts.tile([P, COUT], dt.float32)
        nc.sync.dma_start(b1_s[:], b1_d[:])
        b2_s = consts.tile([P, COUT], dt.float32)
        nc.sync.dma_start(b2_s[:], b2_d[:])

        # h1 slice (local) and allgathered h1 (global), bf16
        hloc = dramp.tile([NPC, COUT], dt.bfloat16)
        hglob = dramp.tile([N, COUT], dt.bfloat16)

        def layer(cin, A_s, is_l1):
            nchunks = cin // ICH
            for w in range(WPC):
                aggw = aggp.tile([P, COUT], dt.float32, tag="aggw")
                # root-term matmul opens the accumulation (start=True)
                if is_l1:
                    lhsTw = rootp.tile([CIN1, P], dt.bfloat16, tag="rootl1")
                    nc.sync.dma_start(lhsTw[:], xT_d[:, w * P:(w + 1) * P])
                    nc.tensor.matmul(aggw[:], lhsT=lhsTw[:], rhs=r1_s[:],
                                     start=True, stop=False)
                else:
                    lhsTw = rootp.tile([CIN2, P], dt.bfloat16, tag="rootl2")
                    nc.sync.dma_start_transpose(
                        lhsTw[:], hloc[w * P:(w + 1) * P, :])
                    nc.tensor.matmul(aggw[:], lhsT=lhsTw[:], rhs=r2_s[:],
                                     start=True, stop=False)
                for t in range(T):
                    e0 = (w * T + t) * P
                    attr_t = meta.tile([3, P], dt.bfloat16, tag="attr")
                    nc.scalar.dma_start(attr_t[:], attrT_d[:, e0:e0 + P])
                    dstt = meta.tile([P, 1], dt.float32, tag="dst")
                    nc.sync.dma_start(dstt[:], dstrel_d[e0:e0 + P, :])

                    cin_t = CIN1 if is_l1 else CIN2
                    xg = xgp.tile([P, cin_t], dt.bfloat16,
                                  tag="xg1" if is_l1 else "xg2")
                    if is_l1:
                        nc.scalar.dma_start(xg[:], xg1_d[e0:e0 + P, :])
                    else:
                        srct = meta.tile([P, 1], dt.int32, tag="src")
                        nc.scalar.dma_start(srct[:], src_d[e0:e0 + P, :])
                        nc.gpsimd.indirect_dma_start(
                            out=xg[:], out_offset=None, in_=hglob[:],
                            in_offset=IndirectOffsetOnAxis(ap=srct[:, :1], axis=0))

                    oh = ohp.tile([P, P], dt.bfloat16, tag="oh")
                    (nc.vector if is_l1 else nc.gpsimd).tensor_scalar(
                        out=oh[:], in0=iota_s[:], scalar1=dstt[:, :1],
                        scalar2=None, op0=mybir.AluOpType.is_equal)

                    # layer 2: chunks 4-7 take the Act/Pool/DVE-reduce
                    # route ((o,i)-permuted A columns, i in [32,64))
                    nalt = 0 if is_l1 else NALT
                    if nalt:
                        msum = msp.tile([P, COUT], dt.bfloat16, tag="ms")
                    ich = min(cin, 2 * ICH)  # 8 (l1) or 16 (l2)
                    ngrp = cin // ich
                    for c in range(ngrp):
                        cols = ich * COUT
                        ppc = pp.tile([P, 2 * ICH * COUT], dt.float32,
                                      tag="ppc")
                        for h in range(0, cols, 512):
                            nc.tensor.matmul(
                                ppc[:, h:h + 512], lhsT=attr_t[:],
                                rhs=A_s[:, c * cols + h:c * cols + h + 512],
                                start=True, stop=True)
                        if True:
                            qc = qp.tile([P, cols], dt.bfloat16,
                                         tag="qc1" if is_l1 else "qc2")
                            q3 = qc[:].rearrange("p (i o) -> p i o", i=ich)
                            nc.vector.scalar_tensor_tensor(
                                out=q3, in0=ppc[:, :cols].rearrange("p (i o) -> p i o", i=ich),
                                scalar=0.0,
                                in1=xg[:, c * ich:(c + 1) * ich].to_broadcast(
                                    [P, ich, COUT]),
                                op0=mybir.AluOpType.max, op1=mybir.AluOpType.mult)
                            for h in range(0, cols, 512):
                                ii = 512 // COUT
                                nc.tensor.matmul(
                                    aggw[:].unsqueeze(1).broadcast_to([P, ii, COUT]),
                                    lhsT=oh[:],
                                    rhs=qc[:, h:h + 512].rearrange(
                                        "p (i o) -> p i o", i=ii),
                                    start=False,
                                    stop=(t == T - 1 and c == ngrp - 1
                                          and h + 512 >= cols and nalt == 0),
                                    skip_group_check=True)
                        else:
                            # (o,i) chunk: o-block of OBL, i in [IALT, 64)
                            cb = c - (nchunks - nalt)
                            ialt = CIN2 - ICH * nalt
                            rc = rp.tile([P, cols], dt.bfloat16, tag="rc")
                            nc.scalar.activation(
                                out=rc[:], in_=ppc[:],
                                func=mybir.ActivationFunctionType.Relu)
                            r3 = rc[:].rearrange("p (o i) -> p o i", o=OBL)
                            qc = qp.tile([P, cols], dt.bfloat16, tag="qa")
                            q3 = qc[:].rearrange("p (o i) -> p o i", o=OBL)
                            nc.gpsimd.tensor_tensor(
                                out=q3, in0=r3,
                                in1=xg[:, ialt:].unsqueeze(1).broadcast_to(
                                    [P, OBL, CIN2 - ialt]),
                                op=mybir.AluOpType.mult)
                            nc.vector.tensor_reduce(
                                out=msum[:, cb * OBL:(cb + 1) * OBL], in_=q3,
                                axis=mybir.AxisListType.X, op=mybir.AluOpType.add)
                    if nalt:
                        nc.tensor.matmul(aggw[:], lhsT=oh[:], rhs=msum[:],
                                         start=False, stop=(t == T - 1),
                                         skip_group_check=True)
                # finalize window: add bias, write out
                if is_l1:
                    hw_ = outp.tile([P, COUT], dt.bfloat16, tag="h1w")
                    nc.vector.tensor_tensor(out=hw_[:], in0=aggw[:], in1=b1_s[:],
                                            op=mybir.AluOpType.add)
                    nc.sync.dma_start(hloc[w * P:(w + 1) * P, :], hw_[:])
                else:
                    ow = outp.tile([P, COUT], dt.float32, tag="outw")
                    nc.vector.tensor_tensor(out=ow[:], in0=aggw[:], in1=b2_s[:],
                                            op=mybir.AluOpType.add)
                    nc.sync.dma_start(out_d[w * P:(w + 1) * P, :], ow[:])

        layer(CIN1, A1_s, True)
        # quarter-AllGathers with contiguous outputs: quarter q of every
        # core's hloc lands at hglob[q*8*qn : (q+1)*8*qn] (core-major inside);
        # the l2 gather indices are remapped host-side to this layout
        CQ = 4
        qn = NPC // CQ
        for q in range(CQ):
            nc.gpsimd.collective_compute(
                "AllGather", mybir.AluOpType.bypass,
                replica_groups=[list(range(NCORES))],
                ins=[hloc[q * qn:(q + 1) * qn, :].opt()],
                outs=[hglob[q * NCORES * qn:(q + 1) * NCORES * qn, :].opt()])
        layer(CIN2, A2_s, False)

    nc.compile()
    return nc


def _pack(edge_index):
    """Relabel nodes into 128 windows of 128 nodes / exactly U edges each.

    Returns (perm, U, order) where perm[orig_node] = new node id and
    order = edge permutation grouping edges by destination window, padded.
    """
    dst = np.asarray(edge_index[1], dtype=np.int64)
    deg = np.bincount(dst, minlength=N).astype(np.int64)
    # LPT greedy: descending degree, least-loaded window with free slots
    nodes = np.argsort(-deg, kind="stable")
    loads = np.zeros(WINDOWS, dtype=np.int64)
    slots = np.zeros(WINDOWS, dtype=np.int64)
    wof = np.empty(N, dtype=np.int64)  # window of node
    for v in nodes:
        open_w = np.flatnonzero(slots < P)
        w = open_w[np.argmin(loads[open_w])]
        wof[v] = w
        loads[w] += deg[v]
        slots[w] += 1
    # repair toward exact target load by swapping nodes between windows
    target = E // WINDOWS
    if loads.max() > target:
        by_wd = {}  # (window, degree) -> list of nodes
        for v in range(N):
            by_wd.setdefault((wof[v], deg[v]), []).append(v)
        for _ in range(100000):
            over = int(np.argmax(loads))
            under = int(np.argmin(loads))
            if loads[over] <= target:
                break
            delta = min(loads[over] - target, target - loads[under])
            # find a swap pair with degree difference = d, largest d first
            done = False
            for d in range(int(delta), 0, -1):
                for da in range(int(deg.max()), d - 1, -1):
                    la = by_wd.get((over, da))
                    lb = by_wd.get((under, da - d))
                    if la and lb:
                        a, b = la.pop(), lb.pop()
                        wof[a], wof[b] = under, over
                        by_wd.setdefault((under, da), []).append(a)
                        by_wd.setdefault((over, da - d), []).append(b)
                        loads[over] -= d
                        loads[under] += d
                        done = True
                        break
                if done:
                    break
            if not done:
                break
    U = int(np.ceil(loads.max() / P) * P)
    # perm: nodes sorted by window -> new ids
    new_order = np.argsort(wof * N + np.arange(N), kind="stable")
    perm = np.empty(N, dtype=np.int64)
    perm[new_order] = np.arange(N)
    # edge order: group by destination window, pad each window to U
    ew = wof[dst]
    eorder = np.argsort(ew, kind="stable")
    counts = np.bincount(ew, minlength=WINDOWS)
    padded = np.full(WINDOWS * U, -1, dtype=np.int64)
    pos = 0
    for w in range(WINDOWS):
        c = int(counts[w])
        padded[w * U:w * U + c] = eorder[pos:pos + c]
        pos += c
    return perm, U, padded


def kernel(x, edge_index, edge_attr, A1, b1, A2, b2, root1, bias1, root2, bias2):
    x = np.asarray(x, dtype=np.float32)
    edge_index = np.asarray(edge_index)
    edge_attr = np.asarray(edge_attr, dtype=np.float32)

    perm, U, padded = _pack(edge_index)
    key = U
    if key not in _cached:
        _cached[key] = _build_program(U)
    nc = _cached[key]

    src = np.asarray(edge_index[0], dtype=np.int64)
    dst = np.asarray(edge_index[1], dtype=np.int64)
    valid = padded >= 0
    pe = np.where(valid, padded, 0)
    # per padded-edge data
    a01 = edge_attr[pe]                      # [W*U, 2]
    aug = valid.astype(np.float32)
    attrT_all = np.stack([a01[:, 0] * aug, a01[:, 1] * aug, aug]).astype(BF16)  # [3, W*U]
    srcn_all = np.where(valid, perm[src[pe]], 0).astype(np.int32)
    dstn = perm[dst[pe]]
    wof_e = np.arange(WINDOWS).repeat(U)
    dstrel_all = np.where(valid, dstn - wof_e * P, 0).astype(np.float32)

    qn = NPC // 4
    sn64 = srcn_all.astype(np.int64)
    src2_all = ((sn64 % NPC) // qn * (NCORES * qn) + (sn64 // NPC) * qn
                + (sn64 % qn)).astype(np.int32)
    x_pi = np.empty_like(x)
    x_pi[perm] = x
    x_bf = x_pi.astype(BF16)
    xg1_all = x_bf[srcn_all]                 # host pre-gather for layer 1

    A1aug = np.concatenate([A1, b1[None, :]], axis=0).astype(BF16)
    A2aug = np.concatenate([A2, b2[None, :]], axis=0).astype(BF16)
    # alt-route chunks cb cover o in [16*cb, 16*cb+16), i in [IALT, 64),
    # laid out o-major / i-minor
    cols = ICH * COUT
    j = np.arange(NALT * cols)
    cb = j // cols
    rem = j % cols
    ol = rem // (CIN2 - IALT)
    i = IALT + rem % (CIN2 - IALT)
    src_cols = i * COUT + (OBL * cb + ol)
    A2perm = A2aug.copy()
    A2perm[:, (8 - NALT) * cols:] = A2aug[:, src_cols]
    iota_np = np.broadcast_to(np.arange(P, dtype=np.float32), (P, P)).astype(BF16)
    b1_bc = np.broadcast_to(bias1, (P, COUT)).astype(np.float32).copy()
    b2_bc = np.broadcast_to(bias2, (P, COUT)).astype(np.float32).copy()
    shared = {
        "A1aug": A1aug, "A2aug": A2perm,
        "root1": np.asarray(root1.astype(BF16)),
        "root2": np.asarray(root2.astype(BF16)),
        "bias1": b1_bc, "bias2": b2_bc,
        "iota": np.asarray(iota_np),
    }
    EPC = WPC * U
    in_maps = []
    for c in range(NCORES):
        s = c * EPC
        m = dict(shared)
        m["attrT"] = attrT_all[:, s:s + EPC].copy()
        m["srcidx"] = src2_all[s:s + EPC].reshape(EPC, 1).copy()
        m["dstrel"] = dstrel_all[s:s + EPC].reshape(EPC, 1).copy()
        m["xg1"] = np.ascontiguousarray(xg1_all[s:s + EPC])
        m["xT"] = np.ascontiguousarray(x_bf[c * NPC:(c + 1) * NPC].T)
        in_maps.append(m)

    res = run_bass_kernel_spmd(nc, in_maps, list(range(NCORES)),
                               **kernel.run_kwargs)
    kernel.last_result = res
    out_pi = np.concatenate([res.results[c]["out"] for c in range(NCORES)], axis=0)
    return out_pi[perm]


kernel.run_kwargs = {}
kernel.last_result = None

